# revision 24
# baseline (speedup 1.0000x reference)
"""ANI-style species-routed MLP (MoE routing) on 8 TRN2 NeuronCores, v2.

Strategy (v2, vs the v1 baseline):
- Atom-level balanced routing: each species' atoms are dealt round-robin
  across the 8 cores, so every core sees ~ceil(N_s/8) atoms of species s and
  the uniform capacity drops to round128(max_s ceil(N_s/8)) (896 here, vs
  960 for per-core molecule sharding).  The per-molecule reduction happens
  on the host, so atoms can be assigned to cores freely.
- Feature-major fp16 matmuls (fp32 PSUM accumulate) as in v1.
- CELU via the exact exp/min/max trick:
      u := celu(z+b)+a = max(z + b + a, min(a*e^{10(z+b)}, a))
                       = min(a*e^{10(z+b)}, a) + relu(z+b)
  (+a offset folded into the next layer's bias on the host, exp's a-scale
  folded into the activation bias).  Per chunk either
      V1: exp on ACT; min (DVE tensor_scalar, 4x fp16) + combine stt on DVE
      V4: exp+relu on ACT; cheap fp16 stt on DVE
  cycled by FORM_PATTERN to balance ACT vs DVE (GPSIMD has no ALU ops on
  TRN2 -- the backend rejects TensorScalarPtr on Pool).
- Partial m-chunks are packed across species to halve their celu cost:
  L2 m1 (64 rows) pairs two species per PSUM tile / celu pass; L3 m1
  (32 rows) packs four.  The packed tiles come from the rotating z pool
  with all their matmuls emitted back-to-back.
- L4 (160->1) matmuls of ALL species write one [7, 512] PSUM tile per
  cap-half (species = partition), so the PSUM->SBUF output copy is 2
  instructions instead of 14.
- All input DMAs are issued up front (species 0 first) and emission is
  software-pipelined across species pairs so the tensor engine always has
  independent work queued and holds its DVFS p-state at full clock.
"""
import os
import sys

sys.path.insert(0, "/opt/trn_rl_repo")

from contextlib import ExitStack

import numpy as np

import concourse.bass as bass
import concourse.mybir as mybir
import concourse.tile as tile
from concourse import bacc
from concourse.bass_utils import run_bass_kernel_spmd

F32 = mybir.dt.float32
F16 = mybir.dt.float16
AF = mybir.ActivationFunctionType
ALU = mybir.AluOpType

B, A, F = 1024, 48, 384
S = 7
NCORES = 8
ALPHA = 0.1
LN_ALPHA = float(np.log(ALPHA))

PAIRS = ((0, 1), (2, 3), (4, 5), (6,))
QUADS = ((0, 1, 2, 3), (4, 5, 6))
SGROUPS = ((0, 1), (2, 3))  # supergroups: pair indices per L3/L4 block

# --- tuning knobs (cache key includes them) ---
FORM_PATTERN = ("V1", "V4", "V1", "V1", "V4", "V1", "V4")
ZBUFS = 4
TBUFS = 6
U1BUFS = 3
U2BUFS = 6
U2P_BUFS = 3
U3BUFS = 4
U3Q_BUFS = 2

_CACHE = {}
LAST_EXEC_NS = None


def _build(cap):
    assert cap % 128 == 0
    halves = [(o, min(512, cap - o)) for o in range(0, cap, 512)]
    nc = bacc.Bacc()

    xt_d = nc.declare_dram_parameter("xt", [128, S, 3, cap], F16, isOutput=False)
    w1_d = nc.declare_dram_parameter("w1t", [128, S, 3, 256], F16, isOutput=False)
    w2_d = nc.declare_dram_parameter("w2t", [128, S, 2, 192], F16, isOutput=False)
    w3_d = nc.declare_dram_parameter("w3t", [128, S, 2, 160], F16, isOutput=False)
    # w4 columns replicated 32x: L4 outputs fill a full 32-partition block
    # (M=32 costs the same as M=1) so the PSUM->SBUF copy reads no stale rows
    w4_d = nc.declare_dram_parameter("w4t", [128, S, 2, 32], F16, isOutput=False)
    # biases [128, S, layer(3), kind(bx,bc,br), m(2)] (m1 slots of L2/L3 unused)
    bb_d = nc.declare_dram_parameter("biases", [128, S, 3, 3, 2], F32,
                                     isOutput=False)
    # packed L2-m1 pair biases [128, npairs, kind]
    bp2_d = nc.declare_dram_parameter("bp2", [128, len(PAIRS), 3], F32,
                                      isOutput=False)
    # packed L3-m1 quad biases [128, nquads, kind]
    bq3_d = nc.declare_dram_parameter("bq3", [128, len(QUADS), 3], F32,
                                      isOutput=False)
    # energy out: partition 32*(s%4), supergroup s//4
    en_d = nc.declare_dram_parameter("energy", [128, len(SGROUPS), cap], F32,
                                     isOutput=True)

    l1_k = [(0, 128), (128, 128), (256, 128)]
    l2_k = [(0, 128), (128, 128)]
    l3_k = [(0, 128), (128, 64)]
    l4_k = [(0, 128), (128, 32)]

    with tile.TileContext(nc) as tc, ExitStack() as ctx:
        wpool = ctx.enter_context(tc.tile_pool(name="weights", bufs=1))
        xpool = ctx.enter_context(tc.tile_pool(name="x", bufs=S))
        u1pool = ctx.enter_context(tc.tile_pool(name="u1", bufs=U1BUFS))
        u2pool = ctx.enter_context(tc.tile_pool(name="u2", bufs=U2BUFS))
        u2ppool = ctx.enter_context(tc.tile_pool(name="u2p", bufs=U2P_BUFS))
        u3pool = ctx.enter_context(tc.tile_pool(name="u3", bufs=U3BUFS))
        u3qpool = ctx.enter_context(tc.tile_pool(name="u3q", bufs=U3Q_BUFS))
        tpool = ctx.enter_context(tc.tile_pool(name="t", bufs=TBUFS))
        zpool = ctx.enter_context(tc.tile_pool(name="z", bufs=ZBUFS, space="PSUM"))
        epool = ctx.enter_context(tc.tile_pool(name="en", bufs=1))

        # --- DMAs: first species prioritized, everything issued up front ---
        bb = wpool.tile([128, S, 3, 3, 2], F32)
        bp2 = wpool.tile([128, len(PAIRS), 3], F32)
        bq3 = wpool.tile([128, len(QUADS), 3], F32)
        nc.sync.dma_start(bb[:], bb_d.ap())
        nc.sync.dma_start(bp2[:], bp2_d.ap())
        nc.sync.dma_start(bq3[:], bq3_d.ap())

        w1 = wpool.tile([128, S, 3, 256], F16)
        w2 = wpool.tile([128, S, 2, 192], F16)
        w3 = wpool.tile([128, S, 2, 160], F16)
        w4 = wpool.tile([128, S, 2, 32], F16)
        x_tiles = {}

        def dma_species(s):
            x_tiles[s] = xpool.tile([128, 3, cap], F16, tag="x", name=f"x{s}")
            nc.sync.dma_start(x_tiles[s][:], xt_d.ap()[:, s])
            nc.sync.dma_start(w1[:, s], w1_d.ap()[:, s])
            nc.sync.dma_start(w2[:, s], w2_d.ap()[:, s])
            nc.sync.dma_start(w3[:, s], w3_d.ap()[:, s])

        dma_species(0)
        nc.sync.dma_start(w4[:], w4_d.ap())
        for s in range(1, S):
            dma_species(s)

        en_sb = epool.tile([128, len(SGROUPS), cap], F32)

        # --- celu ----------------------------------------------------------
        form_idx = 0

        def celu(z, u_out, bx, bc, br):
            """u_out = max(z + bc, min(exp(10 z + bx), alpha)); z PSUM fp32
            view [p, w], u_out SBUF fp16 view [p, w]."""
            nonlocal form_idx
            form = FORM_PATTERN[form_idx % len(FORM_PATTERN)]
            form_idx += 1
            p = z.shape[0]
            w = z.shape[-1]
            e = tpool.tile([128, 1024], F16, tag="e")
            ev = e[:p, :w]
            nc.scalar.activation(ev, z, AF.Exp, bias=bx, scale=10.0)
            if form == "V4":
                r = tpool.tile([128, 1024], F16, tag="r")
                rv = r[:p, :w]
                nc.scalar.activation(rv, z, AF.Relu, bias=br, scale=1.0)
                nc.vector.scalar_tensor_tensor(
                    u_out, ev, ALPHA, rv, op0=ALU.min, op1=ALU.add
                )
            else:
                mt = tpool.tile([128, 1024], F16, tag="mt")
                mv = mt[:p, :w]
                nc.vector.tensor_scalar(mv, ev, ALPHA, None, op0=ALU.min)
                nc.vector.scalar_tensor_tensor(
                    u_out, z, bc, mv, op0=ALU.add, op1=ALU.max
                )

        def celu_s(z, u_out, s, layer, m):
            p = z.shape[0]
            celu(z, u_out,
                 bb[:p, s, layer, 0, m : m + 1],
                 bb[:p, s, layer, 1, m : m + 1],
                 bb[:p, s, layer, 2, m : m + 1])

        # --- layer emitters ------------------------------------------------
        def emit_l1(s):
            u1 = u1pool.tile([128, 2, cap], F16, tag="u1")
            for mi in range(2):
                z = zpool.tile([128, 1024], F32, tag="z")
                for ho, hw in halves:
                    for ki, (ko, kw) in enumerate(l1_k):
                        nc.tensor.matmul(
                            z[:, ho : ho + hw],
                            w1[:, s, ki, mi * 128 : mi * 128 + 128],
                            x_tiles[s][:, ki, ho : ho + hw],
                            start=(ki == 0),
                            stop=(ki == 2),
                        )
                celu_s(z[:, :cap], u1[:, mi, :], s, 0, mi)
            return u1

        def emit_l2m0(s, u1):
            u2 = u2pool.tile([128, cap], F16, tag="u2")
            z = zpool.tile([128, 1024], F32, tag="z")
            for ho, hw in halves:
                for ki, (ko, kw) in enumerate(l2_k):
                    nc.tensor.matmul(
                        z[:, ho : ho + hw],
                        w2[:, s, ki, 0:128],
                        u1[:, ki, ho : ho + hw],
                        start=(ki == 0),
                        stop=(ki == 1),
                    )
            celu_s(z[:, :cap], u2[:], s, 1, 0)
            return u2

        def emit_l2m1(pair, u1s):
            """Pair-packed m1: both species' matmuls back-to-back into one
            rotating z tile, one celu pass."""
            pi = pair[0] // 2
            zp = zpool.tile([128, 1024], F32, tag="z", name="zpair")
            for slot, s in enumerate(pair):
                for ho, hw in halves:
                    for ki, (ko, kw) in enumerate(l2_k):
                        nc.tensor.matmul(
                            zp[64 * slot : 64 * slot + 64, ho : ho + hw],
                            w2[:, s, ki, 128:192],
                            u1s[s][:, ki, ho : ho + hw],
                            start=(ki == 0),
                            stop=(ki == 1),
                        )
            u2p = u2ppool.tile([128, cap], F16, tag="u2p")
            npart = 64 * len(pair)
            celu(zp[:npart, :cap], u2p[:npart, :],
                 bp2[:npart, pi, 0:1], bp2[:npart, pi, 1:2],
                 bp2[:npart, pi, 2:3])
            return u2p

        def emit_l3m0(s, u2m0, u2p, slot):
            u3 = u3pool.tile([128, cap], F16, tag="u3")
            z = zpool.tile([128, 1024], F32, tag="z")
            po = 64 * slot
            for ho, hw in halves:
                for ki, (ko, kw) in enumerate(l3_k):
                    if ki == 0:
                        lhsT = w3[:, s, 0, 0:128]
                        rhs = u2m0[:, ho : ho + hw]
                    else:
                        lhsT = w3[po : po + 64, s, 1, 0:128]
                        rhs = u2p[po : po + 64, ho : ho + hw]
                    nc.tensor.matmul(
                        z[:, ho : ho + hw],
                        lhsT,
                        rhs,
                        start=(ki == 0),
                        stop=(ki == 1),
                    )
            celu_s(z[:, :cap], u3[:], s, 2, 0)
            return u3

        def emit_l3q(quad, u2m0s, u2pairs):
            """Quad-packed L3 m1 (32 rows each) into one z tile."""
            qi = quad[0] // 4
            zq = zpool.tile([128, 1024], F32, tag="z", name="zquad")
            for j, s in enumerate(quad):
                po = 64 * (s % 2)
                u2p = u2pairs[s // 2]
                for ho, hw in halves:
                    for ki, (ko, kw) in enumerate(l3_k):
                        if ki == 0:
                            lhsT = w3[:, s, 0, 128:160]
                            rhs = u2m0s[s][:, ho : ho + hw]
                            tp = (0, 32 * j)
                        else:
                            lhsT = w3[po : po + 64, s, 1, 128:160]
                            rhs = u2p[po : po + 64, ho : ho + hw]
                            tp = (po, 32 * j)
                        nc.tensor.matmul(
                            zq[32 * j : 32 * j + 32, ho : ho + hw],
                            lhsT,
                            rhs,
                            start=(ki == 0),
                            stop=(ki == 1),
                            tile_position=tp,
                        )
            u3q = u3qpool.tile([128, cap], F16, tag="u3q")
            npart = 32 * len(quad)
            celu(zq[:npart, :cap], u3q[:npart, :],
                 bq3[:npart, qi, 0:1], bq3[:npart, qi, 1:2],
                 bq3[:npart, qi, 2:3])
            return u3q

        def emit_l4(s, u3m0, u3q, j, z4):
            qo = 32 * j
            for ho, hw in halves:
                for ki, (ko, kw) in enumerate(l4_k):
                    if ki == 0:
                        lhsT = w4[:, s, 0, :]
                        rhs = u3m0[:, ho : ho + hw]
                        tp = (0, 32 * j)
                    else:
                        lhsT = w4[qo : qo + 32, s, 1, :]
                        rhs = u3q[qo : qo + 32, ho : ho + hw]
                        tp = (qo, 32 * j)
                    nc.tensor.matmul(
                        z4[32 * j : 32 * j + 32, ho : ho + hw],
                        lhsT,
                        rhs,
                        start=(ki == 0),
                        stop=(ki == 1),
                        tile_position=tp,
                    )

        # --- emission: pair fronts pipelined with supergroup L3/L4 ----------
        u1s, u2m0s, u2pairs, u3m0s, u3qs = {}, {}, {}, {}, {}

        def emit_front(pair):
            for s in pair:
                u1s[s] = emit_l1(s)
                u2m0s[s] = emit_l2m0(s, u1s[s])
            u2pairs[pair[0] // 2] = emit_l2m1(pair, u1s)

        def emit_tail(gi):
            quad = QUADS[gi]
            for s in quad:
                u3m0s[s] = emit_l3m0(s, u2m0s[s], u2pairs[s // 2], s % 2)
            u3qs[gi] = emit_l3q(quad, u2m0s, u2pairs)
            z4 = zpool.tile([128, 1024], F32, tag="z", name="z4")
            for j, s in enumerate(quad):
                emit_l4(s, u3m0s[s], u3qs[gi], j, z4)
            # one PSUM->SBUF copy for the whole supergroup (species live at
            # partitions 0/32/64/96; halves at cols 0-511 / 512-...)
            np_ = 32 * len(quad)
            if gi % 2 == 0:
                nc.scalar.copy(en_sb[:np_, gi, :cap], z4[:np_, :cap])
            else:
                nc.vector.tensor_copy(en_sb[:np_, gi, :cap], z4[:np_, :cap])

        emit_front(PAIRS[0])
        emit_front(PAIRS[1])
        emit_front(PAIRS[2])
        emit_tail(0)
        emit_front(PAIRS[3])
        emit_tail(1)

        nc.sync.dma_start(en_d.ap()[:, 0], en_sb[:, 0])
        nc.sync.dma_start(en_d.ap()[: 32 * len(QUADS[1]), 1],
                          en_sb[: 32 * len(QUADS[1]), 1])

    nc.compile()
    return nc


def _to_pmajor(wt, k_pad):
    """[S, M, K] weights -> [128, S, k_pad//128, M] fp16 partition-major."""
    s, m, k = wt.shape
    arr = np.zeros((s, m, k_pad), np.float32)
    arr[:, :, :k] = wt
    out = arr.transpose(2, 0, 1).reshape(k_pad // 128, 128, s, m).transpose(1, 2, 0, 3)
    return np.ascontiguousarray(out, dtype=np.float16)


def _prep_weights(W1, b1, W2, b2, W3, b3, W4, b4):
    beta1 = b1
    beta2 = b2 - ALPHA * W2.sum(axis=2)
    beta3 = b3 - ALPHA * W3.sum(axis=2)
    ec = (b4[:, 0] - ALPHA * W4[:, 0, :].sum(axis=1)).astype(np.float64)

    def kinds(beta):
        return (10.0 * beta + LN_ALPHA, beta + ALPHA, beta)

    bb = np.zeros((128, S, 3, 3, 2), np.float32)
    for li, beta in enumerate((beta1, beta2, beta3)):
        m = beta.shape[1]
        pad = np.zeros((S, 256), np.float32)
        pad[:, :m] = beta
        for k, arr in enumerate(kinds(pad)):
            for mi in range(2):
                bb[:, :, li, k, mi] = arr[:, mi * 128 : mi * 128 + 128].T

    bp2 = np.zeros((128, len(PAIRS), 3), np.float32)
    for pi, pair in enumerate(PAIRS):
        for slot, s in enumerate(pair):
            sl = slice(64 * slot, 64 * slot + 64)
            for k, arr in enumerate(kinds(beta2[s][128:192])):
                bp2[sl, pi, k] = arr

    bq3 = np.zeros((128, len(QUADS), 3), np.float32)
    for qi, quad in enumerate(QUADS):
        for j, s in enumerate(quad):
            sl = slice(32 * j, 32 * j + 32)
            for k, arr in enumerate(kinds(beta3[s][128:160])):
                bq3[sl, qi, k] = arr

    # w3/w4 k-tile1 replicated so packed slots can slice at their own base
    # partition (matmul requires lhsT/rhs base partitions to match); w4 also
    # replicated 32x along M so L4 fills full 32-partition output blocks
    w3t = _to_pmajor(W3, 256)
    w3t[64:128, :, 1, :] = w3t[0:64, :, 1, :]
    w4t1 = _to_pmajor(W4, 256)  # [128, S, 2, 1]
    w4t = np.ascontiguousarray(np.broadcast_to(w4t1, (128, S, 2, 32)))
    w4t = w4t.copy()
    for j in range(1, 4):
        w4t[32 * j : 32 * j + 32, :, 1, :] = w4t[0:32, :, 1, :]

    return dict(
        w1t=_to_pmajor(W1, 384),
        w2t=_to_pmajor(W2, 256),
        w3t=w3t,
        w4t=w4t,
        biases=bb, bp2=bp2, bq3=bq3,
    ), ec


def kernel(species, aev, W1, b1, W2, b2, W3, b3, W4, b4):
    global LAST_EXEC_NS
    species = np.asarray(species)
    aev = np.asarray(aev, dtype=np.float32)
    args = [np.asarray(x, dtype=np.float32)
            for x in (W1, b1, W2, b2, W3, b3, W4, b4)]

    sp = species.reshape(-1)
    aev_f = aev.reshape(-1, F)

    # --- balanced atom routing: deal each species round-robin to cores ---
    idx_by_s = [np.nonzero(sp == s)[0] for s in range(S)]
    core_lists = [[idx_by_s[s][c::NCORES] for s in range(S)]
                  for c in range(NCORES)]
    max_n = max(len(core_lists[c][s]) for c in range(NCORES) for s in range(S))
    cap = int(((max_n + 127) // 128) * 128)

    wp, ec = _prep_weights(*args)

    key = (cap, FORM_PATTERN, ZBUFS, TBUFS, U1BUFS, U2BUFS, U2P_BUFS, U3BUFS,
           U3Q_BUFS)
    if key not in _CACHE:
        _CACHE[key] = _build(cap)
    nc = _CACHE[key]

    in_maps = []
    for c in range(NCORES):
        xt = np.zeros((128, S, 3, cap), np.float16)
        for s in range(S):
            idx = core_lists[c][s]
            n = len(idx)
            blk = aev_f[idx].T.astype(np.float16)  # [384, n]
            xt[:, s, :, :n] = blk.reshape(3, 128, n).transpose(1, 0, 2)
        in_maps.append({"xt": xt, **wp})

    trace = bool(os.environ.get("KERNEL_TRACE"))
    res = run_bass_kernel_spmd(nc, in_maps, list(range(NCORES)), trace=trace)
    LAST_EXEC_NS = res.exec_time_ns

    # --- host reduction ---
    atom_e = np.empty(B * A, np.float64)
    for c in range(NCORES):
        en = np.asarray(res.results[c]["energy"], np.float64)  # [128, 2, cap]
        for s in range(S):
            idx = core_lists[c][s]
            atom_e[idx] = en[32 * (s % 4), s // 4, : len(idx)] + ec[s]
    return atom_e.reshape(B, A).sum(axis=1).astype(np.float32)


# revision 29
# speedup vs baseline: 1.1146x; 1.1146x over previous
"""ANI-style species-routed MLP (MoE routing) on 8 TRN2 NeuronCores, v2.

Strategy (v2, vs the v1 baseline):
- Atom-level balanced routing: each species' atoms are dealt round-robin
  across the 8 cores, so every core sees ~ceil(N_s/8) atoms of species s and
  the uniform capacity drops to round128(max_s ceil(N_s/8)) (896 here, vs
  960 for per-core molecule sharding).  The per-molecule reduction happens
  on the host, so atoms can be assigned to cores freely.
- Feature-major fp16 matmuls (fp32 PSUM accumulate) as in v1.
- CELU via the exact exp/min/max trick:
      u := celu(z+b)+a = max(z + b + a, min(a*e^{10(z+b)}, a))
                       = min(a*e^{10(z+b)}, a) + relu(z+b)
  (+a offset folded into the next layer's bias on the host, exp's a-scale
  folded into the activation bias).  Per chunk either
      V1: exp on ACT; min (DVE tensor_scalar, 4x fp16) + combine stt on DVE
      V4: exp+relu on ACT; cheap fp16 stt on DVE
  cycled by FORM_PATTERN to balance ACT vs DVE (GPSIMD has no ALU ops on
  TRN2 -- the backend rejects TensorScalarPtr on Pool).
- Partial m-chunks are packed across species to halve their celu cost:
  L2 m1 (64 rows) pairs two species per PSUM tile / celu pass; L3 m1
  (32 rows) packs four.  The packed tiles come from the rotating z pool
  with all their matmuls emitted back-to-back.
- L4 (160->1) matmuls of ALL species write one [7, 512] PSUM tile per
  cap-half (species = partition), so the PSUM->SBUF output copy is 2
  instructions instead of 14.
- All input DMAs are issued up front (species 0 first) and emission is
  software-pipelined across species pairs so the tensor engine always has
  independent work queued and holds its DVFS p-state at full clock.
"""
import os
import sys

sys.path.insert(0, "/opt/trn_rl_repo")

from contextlib import ExitStack

import numpy as np

import concourse.bass as bass
import concourse.mybir as mybir
import concourse.tile as tile
from concourse import bacc
from concourse.bass_utils import run_bass_kernel_spmd

F32 = mybir.dt.float32
F16 = mybir.dt.float16
AF = mybir.ActivationFunctionType
ALU = mybir.AluOpType

B, A, F = 1024, 48, 384
S = 7
NCORES = 8
ALPHA = 0.1
LN_ALPHA = float(np.log(ALPHA))

PAIRS = ((0, 1), (2, 3), (4, 5), (6,))
QUADS = ((0, 1, 2, 3), (4, 5, 6))
SGROUPS = ((0, 1), (2, 3))  # supergroups: pair indices per L3/L4 block

# --- tuning knobs (cache key includes them) ---
FORM_PATTERN = ("V1", "V4", "V1", "V1", "V4", "V1", "V4")
ZBUFS = 4
TBUFS = 6
U1BUFS = 3
U2BUFS = 7
U2P_BUFS = 4
U3BUFS = 4
U3Q_BUFS = 2

_CACHE = {}
LAST_EXEC_NS = None


def _build(cap):
    assert cap % 128 == 0
    halves = [(o, min(512, cap - o)) for o in range(0, cap, 512)]
    nc = bacc.Bacc()

    xt_d = nc.declare_dram_parameter("xt", [128, S, 3, cap], F16, isOutput=False)
    w1_d = nc.declare_dram_parameter("w1t", [128, S, 3, 256], F16, isOutput=False)
    w2_d = nc.declare_dram_parameter("w2t", [128, S, 2, 192], F16, isOutput=False)
    w3_d = nc.declare_dram_parameter("w3t", [128, S, 2, 160], F16, isOutput=False)
    # w4 columns replicated 32x: L4 outputs fill a full 32-partition block
    # (M=32 costs the same as M=1) so the PSUM->SBUF copy reads no stale rows
    w4_d = nc.declare_dram_parameter("w4t", [128, S, 2, 32], F16, isOutput=False)
    # biases [128, S, layer(3), kind(bx,bc,br), m(2)] (m1 slots of L2/L3 unused)
    bb_d = nc.declare_dram_parameter("biases", [128, S, 3, 3, 2], F32,
                                     isOutput=False)
    # packed L2-m1 pair biases [128, npairs, kind]
    bp2_d = nc.declare_dram_parameter("bp2", [128, len(PAIRS), 3], F32,
                                      isOutput=False)
    # packed L3-m1 quad biases [128, nquads, kind]
    bq3_d = nc.declare_dram_parameter("bq3", [128, len(QUADS), 3], F32,
                                      isOutput=False)
    # energy out: partition 32*(s%4), supergroup s//4
    en_d = nc.declare_dram_parameter("energy", [128, len(SGROUPS), cap], F32,
                                     isOutput=True)

    l1_k = [(0, 128), (128, 128), (256, 128)]
    l2_k = [(0, 128), (128, 128)]
    l3_k = [(0, 128), (128, 64)]
    l4_k = [(0, 128), (128, 32)]

    with tile.TileContext(nc) as tc, ExitStack() as ctx:
        wpool = ctx.enter_context(tc.tile_pool(name="weights", bufs=1))
        xpool = ctx.enter_context(tc.tile_pool(name="x", bufs=S))
        u1pool = ctx.enter_context(tc.tile_pool(name="u1", bufs=U1BUFS))
        u2pool = ctx.enter_context(tc.tile_pool(name="u2", bufs=U2BUFS))
        u2ppool = ctx.enter_context(tc.tile_pool(name="u2p", bufs=U2P_BUFS))
        u3pool = ctx.enter_context(tc.tile_pool(name="u3", bufs=U3BUFS))
        u3qpool = ctx.enter_context(tc.tile_pool(name="u3q", bufs=U3Q_BUFS))
        tpool = ctx.enter_context(tc.tile_pool(name="t", bufs=TBUFS))
        zpool = ctx.enter_context(tc.tile_pool(name="z", bufs=ZBUFS, space="PSUM"))
        epool = ctx.enter_context(tc.tile_pool(name="en", bufs=1))

        # --- DMAs: first species prioritized, everything issued up front ---
        bb = wpool.tile([128, S, 3, 3, 2], F32)
        bp2 = wpool.tile([128, len(PAIRS), 3], F32)
        bq3 = wpool.tile([128, len(QUADS), 3], F32)
        nc.sync.dma_start(bb[:], bb_d.ap())
        nc.sync.dma_start(bp2[:], bp2_d.ap())
        nc.sync.dma_start(bq3[:], bq3_d.ap())

        w1 = wpool.tile([128, S, 3, 256], F16)
        w2 = wpool.tile([128, S, 2, 192], F16)
        w3 = wpool.tile([128, S, 2, 160], F16)
        w4 = wpool.tile([128, S, 2, 32], F16)
        x_tiles = {}

        def dma_species(s):
            x_tiles[s] = xpool.tile([128, 3, cap], F16, tag="x", name=f"x{s}")
            nc.sync.dma_start(w1[:, s], w1_d.ap()[:, s])
            for k in range(3):  # per-k-chunk so L1 can start on chunk 0
                nc.sync.dma_start(x_tiles[s][:, k], xt_d.ap()[:, s, k])
            nc.sync.dma_start(w2[:, s], w2_d.ap()[:, s])
            nc.sync.dma_start(w3[:, s], w3_d.ap()[:, s])

        dma_species(0)
        nc.sync.dma_start(w4[:], w4_d.ap())
        for s in range(1, S):
            dma_species(s)

        en_sb = epool.tile([128, len(SGROUPS), cap], F32)

        # --- celu ----------------------------------------------------------
        form_idx = 0

        def celu(z, u_out, bx, bc, br):
            """u_out = max(z + bc, min(exp(10 z + bx), alpha)); z PSUM fp32
            view [p, w], u_out SBUF fp16 view [p, w]."""
            nonlocal form_idx
            form = FORM_PATTERN[form_idx % len(FORM_PATTERN)]
            form_idx += 1
            p = z.shape[0]
            w = z.shape[-1]
            e = tpool.tile([128, 1024], F16, tag="e")
            ev = e[:p, :w]
            nc.scalar.activation(ev, z, AF.Exp, bias=bx, scale=10.0)
            if form == "V4":
                r = tpool.tile([128, 1024], F16, tag="r")
                rv = r[:p, :w]
                nc.scalar.activation(rv, z, AF.Relu, bias=br, scale=1.0)
                nc.vector.scalar_tensor_tensor(
                    u_out, ev, ALPHA, rv, op0=ALU.min, op1=ALU.add
                )
            else:
                mt = tpool.tile([128, 1024], F16, tag="mt")
                mv = mt[:p, :w]
                nc.vector.tensor_scalar(mv, ev, ALPHA, None, op0=ALU.min)
                nc.vector.scalar_tensor_tensor(
                    u_out, z, bc, mv, op0=ALU.add, op1=ALU.max
                )

        def celu_s(z, u_out, s, layer, m):
            p = z.shape[0]
            celu(z, u_out,
                 bb[:p, s, layer, 0, m : m + 1],
                 bb[:p, s, layer, 1, m : m + 1],
                 bb[:p, s, layer, 2, m : m + 1])

        # --- layer emitters ------------------------------------------------
        def emit_l1(s):
            u1 = u1pool.tile([128, 2, cap], F16, tag="u1")
            for mi in range(2):
                z = zpool.tile([128, 1024], F32, tag="z")
                for ho, hw in halves:
                    for ki, (ko, kw) in enumerate(l1_k):
                        nc.tensor.matmul(
                            z[:, ho : ho + hw],
                            w1[:, s, ki, mi * 128 : mi * 128 + 128],
                            x_tiles[s][:, ki, ho : ho + hw],
                            start=(ki == 0),
                            stop=(ki == 2),
                        )
                celu_s(z[:, :cap], u1[:, mi, :], s, 0, mi)
            return u1

        def emit_l2m0(s, u1):
            u2 = u2pool.tile([128, cap], F16, tag="u2")
            z = zpool.tile([128, 1024], F32, tag="z")
            for ho, hw in halves:
                for ki, (ko, kw) in enumerate(l2_k):
                    nc.tensor.matmul(
                        z[:, ho : ho + hw],
                        w2[:, s, ki, 0:128],
                        u1[:, ki, ho : ho + hw],
                        start=(ki == 0),
                        stop=(ki == 1),
                    )
            celu_s(z[:, :cap], u2[:], s, 1, 0)
            return u2

        def emit_l2m1_mm(s, slot, u1, zp):
            for ho, hw in halves:
                for ki, (ko, kw) in enumerate(l2_k):
                    nc.tensor.matmul(
                        zp[64 * slot : 64 * slot + 64, ho : ho + hw],
                        w2[:, s, ki, 128:192],
                        u1[:, ki, ho : ho + hw],
                        start=(ki == 0),
                        stop=(ki == 1),
                    )

        def emit_l2m1_celu(pair, zp):
            pi = pair[0] // 2
            u2p = u2ppool.tile([128, cap], F16, tag="u2p")
            npart = 64 * len(pair)
            celu(zp[:npart, :cap], u2p[:npart, :],
                 bp2[:npart, pi, 0:1], bp2[:npart, pi, 1:2],
                 bp2[:npart, pi, 2:3])
            return u2p

        def emit_l3m0(s, u2m0, u2p, slot):
            u3 = u3pool.tile([128, cap], F16, tag="u3")
            z = zpool.tile([128, 1024], F32, tag="z")
            po = 64 * slot
            for ho, hw in halves:
                for ki, (ko, kw) in enumerate(l3_k):
                    if ki == 0:
                        lhsT = w3[:, s, 0, 0:128]
                        rhs = u2m0[:, ho : ho + hw]
                    else:
                        lhsT = w3[po : po + 64, s, 1, 0:128]
                        rhs = u2p[po : po + 64, ho : ho + hw]
                    nc.tensor.matmul(
                        z[:, ho : ho + hw],
                        lhsT,
                        rhs,
                        start=(ki == 0),
                        stop=(ki == 1),
                    )
            celu_s(z[:, :cap], u3[:], s, 2, 0)
            return u3

        def emit_l3q(quad, u2m0s, u2pairs):
            """Quad-packed L3 m1 (32 rows each) into one z tile."""
            qi = quad[0] // 4
            zq = zpool.tile([128, 1024], F32, tag="z", name="zquad")
            for j, s in enumerate(quad):
                po = 64 * (s % 2)
                u2p = u2pairs[s // 2]
                for ho, hw in halves:
                    for ki, (ko, kw) in enumerate(l3_k):
                        if ki == 0:
                            lhsT = w3[:, s, 0, 128:160]
                            rhs = u2m0s[s][:, ho : ho + hw]
                            tp = (0, 32 * j)
                        else:
                            lhsT = w3[po : po + 64, s, 1, 128:160]
                            rhs = u2p[po : po + 64, ho : ho + hw]
                            tp = (po, 32 * j)
                        nc.tensor.matmul(
                            zq[32 * j : 32 * j + 32, ho : ho + hw],
                            lhsT,
                            rhs,
                            start=(ki == 0),
                            stop=(ki == 1),
                            tile_position=tp,
                        )
            u3q = u3qpool.tile([128, cap], F16, tag="u3q")
            npart = 32 * len(quad)
            celu(zq[:npart, :cap], u3q[:npart, :],
                 bq3[:npart, qi, 0:1], bq3[:npart, qi, 1:2],
                 bq3[:npart, qi, 2:3])
            return u3q

        def emit_l4(s, u3m0, u3q, j, z4):
            qo = 32 * j
            for ho, hw in halves:
                for ki, (ko, kw) in enumerate(l4_k):
                    if ki == 0:
                        lhsT = w4[:, s, 0, :]
                        rhs = u3m0[:, ho : ho + hw]
                        tp = (0, 32 * j)
                    else:
                        lhsT = w4[qo : qo + 32, s, 1, :]
                        rhs = u3q[qo : qo + 32, ho : ho + hw]
                        tp = (qo, 32 * j)
                    nc.tensor.matmul(
                        z4[32 * j : 32 * j + 32, ho : ho + hw],
                        lhsT,
                        rhs,
                        start=(ki == 0),
                        stop=(ki == 1),
                        tile_position=tp,
                    )

        # --- emission: pair fronts pipelined with supergroup L3/L4 ----------
        u1s, u2m0s, u2pairs, u3m0s, u3qs = {}, {}, {}, {}, {}

        def emit_front(pair):
            # all L1 matmuls before any L2 so the in-order PE stream never
            # waits on an L1 celu that was emitted moments earlier; the pair
            # m1 matmuls of species A pad the gap before species B's L2
            for s in pair:
                u1s[s] = emit_l1(s)
            zp = zpool.tile([128, 1024], F32, tag="z", name="zpair")
            for slot, s in enumerate(pair):
                u2m0s[s] = emit_l2m0(s, u1s[s])
                emit_l2m1_mm(s, slot, u1s[s], zp)
            u2pairs[pair[0] // 2] = emit_l2m1_celu(pair, zp)

        def emit_tail(gi):
            quad = QUADS[gi]
            # quad first: its celu latency is covered by the L3m0 matmuls
            u3qs[gi] = emit_l3q(quad, u2m0s, u2pairs)
            for s in quad:
                u3m0s[s] = emit_l3m0(s, u2m0s[s], u2pairs[s // 2], s % 2)
            z4 = zpool.tile([128, 1024], F32, tag="z", name="z4")
            for j, s in enumerate(quad):
                emit_l4(s, u3m0s[s], u3qs[gi], j, z4)
            # one PSUM->SBUF copy for the whole supergroup (species live at
            # partition blocks 32j; halves at cols 0-511 / 512-...)
            np_ = 32 * len(quad)
            if gi % 2 == 0:
                nc.scalar.copy(en_sb[:np_, gi, :cap], z4[:np_, :cap])
            else:
                nc.vector.tensor_copy(en_sb[:np_, gi, :cap], z4[:np_, :cap])

        emit_front(PAIRS[0])
        emit_front(PAIRS[1])
        emit_front(PAIRS[2])
        emit_front(PAIRS[3])
        emit_tail(0)
        emit_tail(1)

        nc.sync.dma_start(en_d.ap()[:, 0], en_sb[:, 0])
        nc.sync.dma_start(en_d.ap()[: 32 * len(QUADS[1]), 1],
                          en_sb[: 32 * len(QUADS[1]), 1])

    nc.compile()
    return nc


def _to_pmajor(wt, k_pad):
    """[S, M, K] weights -> [128, S, k_pad//128, M] fp16 partition-major."""
    s, m, k = wt.shape
    arr = np.zeros((s, m, k_pad), np.float32)
    arr[:, :, :k] = wt
    out = arr.transpose(2, 0, 1).reshape(k_pad // 128, 128, s, m).transpose(1, 2, 0, 3)
    return np.ascontiguousarray(out, dtype=np.float16)


def _prep_weights(W1, b1, W2, b2, W3, b3, W4, b4):
    beta1 = b1
    beta2 = b2 - ALPHA * W2.sum(axis=2)
    beta3 = b3 - ALPHA * W3.sum(axis=2)
    ec = (b4[:, 0] - ALPHA * W4[:, 0, :].sum(axis=1)).astype(np.float64)

    def kinds(beta):
        return (10.0 * beta + LN_ALPHA, beta + ALPHA, beta)

    bb = np.zeros((128, S, 3, 3, 2), np.float32)
    for li, beta in enumerate((beta1, beta2, beta3)):
        m = beta.shape[1]
        pad = np.zeros((S, 256), np.float32)
        pad[:, :m] = beta
        for k, arr in enumerate(kinds(pad)):
            for mi in range(2):
                bb[:, :, li, k, mi] = arr[:, mi * 128 : mi * 128 + 128].T

    bp2 = np.zeros((128, len(PAIRS), 3), np.float32)
    for pi, pair in enumerate(PAIRS):
        for slot, s in enumerate(pair):
            sl = slice(64 * slot, 64 * slot + 64)
            for k, arr in enumerate(kinds(beta2[s][128:192])):
                bp2[sl, pi, k] = arr

    bq3 = np.zeros((128, len(QUADS), 3), np.float32)
    for qi, quad in enumerate(QUADS):
        for j, s in enumerate(quad):
            sl = slice(32 * j, 32 * j + 32)
            for k, arr in enumerate(kinds(beta3[s][128:160])):
                bq3[sl, qi, k] = arr

    # w3/w4 k-tile1 replicated so packed slots can slice at their own base
    # partition (matmul requires lhsT/rhs base partitions to match); w4 also
    # replicated 32x along M so L4 fills full 32-partition output blocks
    w3t = _to_pmajor(W3, 256)
    w3t[64:128, :, 1, :] = w3t[0:64, :, 1, :]
    w4t1 = _to_pmajor(W4, 256)  # [128, S, 2, 1]
    w4t = np.ascontiguousarray(np.broadcast_to(w4t1, (128, S, 2, 32)))
    w4t = w4t.copy()
    for j in range(1, 4):
        w4t[32 * j : 32 * j + 32, :, 1, :] = w4t[0:32, :, 1, :]

    return dict(
        w1t=_to_pmajor(W1, 384),
        w2t=_to_pmajor(W2, 256),
        w3t=w3t,
        w4t=w4t,
        biases=bb, bp2=bp2, bq3=bq3,
    ), ec


def kernel(species, aev, W1, b1, W2, b2, W3, b3, W4, b4):
    global LAST_EXEC_NS
    species = np.asarray(species)
    aev = np.asarray(aev, dtype=np.float32)
    args = [np.asarray(x, dtype=np.float32)
            for x in (W1, b1, W2, b2, W3, b3, W4, b4)]

    sp = species.reshape(-1)
    aev_f = aev.reshape(-1, F)

    # --- balanced atom routing: deal each species round-robin to cores ---
    idx_by_s = [np.nonzero(sp == s)[0] for s in range(S)]
    core_lists = [[idx_by_s[s][c::NCORES] for s in range(S)]
                  for c in range(NCORES)]
    max_n = max(len(core_lists[c][s]) for c in range(NCORES) for s in range(S))
    cap = int(((max_n + 127) // 128) * 128)

    wp, ec = _prep_weights(*args)

    key = (cap, FORM_PATTERN, ZBUFS, TBUFS, U1BUFS, U2BUFS, U2P_BUFS, U3BUFS,
           U3Q_BUFS)
    if key not in _CACHE:
        _CACHE[key] = _build(cap)
    nc = _CACHE[key]

    in_maps = []
    for c in range(NCORES):
        xt = np.zeros((128, S, 3, cap), np.float16)
        for s in range(S):
            idx = core_lists[c][s]
            n = len(idx)
            blk = aev_f[idx].T.astype(np.float16)  # [384, n]
            xt[:, s, :, :n] = blk.reshape(3, 128, n).transpose(1, 0, 2)
        in_maps.append({"xt": xt, **wp})

    trace = bool(os.environ.get("KERNEL_TRACE"))
    res = run_bass_kernel_spmd(nc, in_maps, list(range(NCORES)), trace=trace)
    LAST_EXEC_NS = res.exec_time_ns

    # --- host reduction ---
    atom_e = np.empty(B * A, np.float64)
    for c in range(NCORES):
        en = np.asarray(res.results[c]["energy"], np.float64)  # [128, 2, cap]
        for s in range(S):
            idx = core_lists[c][s]
            atom_e[idx] = en[32 * (s % 4), s // 4, : len(idx)] + ec[s]
    return atom_e.reshape(B, A).sum(axis=1).astype(np.float32)
